# revision 11
# baseline (speedup 1.0000x reference)
"""LSTM (B=131072, T=10, INP=HID=64) + linear head, data-parallel on 8 TRN2 cores.

v6 layout (per core, B_loc=16384, 16 "units" of two 512-col groups A/B):
  - Feature-major: features on SBUF partitions, batch on the free dim. PSUM
    per unit-step: [128, 4, NB] banks (i, f, o, g), bank = [gate_A(0:64);
    gate_B(64:128)], so all elementwise ops run 128 lanes wide.
  - Both groups use rhs layout [h(0:64); x(64:128)] inside one persistent
    tile RAB[128, group, slot(4), NB] per unit, sharing a single weight copy.
    h_A lands aligned; h_B is written with a cross-partition output (legal:
    only tensor-op *inputs* must share a base partition). x is DMA'd two
    steps ahead into slot pairs, so DMA WAR waits are against long-retired
    readers and the SP queue never head-of-line blocks.
  - Bias: banks 0-1 seeded by K=1 matmuls on PE, banks 2-3 by one DVE copy
    from an SBUF bias image; gate matmuls accumulate on top. One merged
    sigmoid covers all 4 banks (g weights/bias pre-doubled so
    tanh(g) = 2*sig(2g)-1); tanh(c) batched across unit pairs.
  - Elementwise: i*g and f*c products on Pool(GPSIMD); Gt fix, c-add, h-muls,
    head staging on DVE.
"""

import numpy as np
import ml_dtypes

import concourse.bass as bass
import concourse.mybir as mybir
from concourse import bacc
import concourse.tile as tile

HID = 64
INP = 64
T = 10
B = 131072
NCORES = 8
B_LOC = B // NCORES   # 16384
NB = 512              # batch columns per group
NUNITS = B_LOC // (2 * NB)  # 16 units of (A, B) groups
NSLOT = 4             # rhs time slots (2-step DMA chunks, 2-step prefetch)

BF = mybir.dt.bfloat16
F32 = mybir.dt.float32
AF = mybir.ActivationFunctionType
ALU = mybir.AluOpType

# psum gate-slice order: 0=i, 1=f, 2=o, 3=g ; torch block order i,f,g,o
SLICE_TO_TORCH_GATE = [0, 1, 3, 2]


def emit_lstm(tc, aps):
    nc = tc.nc
    xab, Wd, BWd, BId, WOd, BOd, y = (
        aps["xab"], aps["Wd"], aps["BWd"], aps["BId"], aps["WOd"], aps["BOd"],
        aps["y"])

    with (
        tc.tile_pool(name="const", bufs=1) as cpool,
        tc.tile_pool(name="rhs", bufs=1) as rpool,
        tc.tile_pool(name="cstate", bufs=2) as spool,
        tc.tile_pool(name="gs", bufs=4) as gpool,
        tc.tile_pool(name="work", bufs=2) as wpool,
        tc.tile_pool(name="small", bufs=3) as qpool,
        tc.tile_pool(name="psum", bufs=2, space="PSUM") as ppool,
    ):
        W = cpool.tile([128, 4, 64], BF)    # [k(h;x), slice, m]
        nc.sync.dma_start(out=W, in_=Wd)
        BW = cpool.tile([1, 4, 128], BF)
        nc.sync.dma_start(out=BW, in_=BWd)
        BI = cpool.tile([128, 2, NB], BF)   # bias image for banks 2-3 (o, g)
        nc.sync.dma_start(out=BI, in_=BId)
        WO = cpool.tile([64, 1], BF)
        nc.sync.dma_start(out=WO, in_=WOd)
        BO = cpool.tile([1, 1], BF)
        nc.sync.dma_start(out=BO, in_=BOd)
        ones_sb = cpool.tile([1, NB], BF)
        nc.vector.memset(ones_sb, 1.0)

        # persistent rhs tiles: [h(0:64); x(64:128)] x group x slot
        R = [rpool.tile([128, 2, NSLOT, NB], BF, tag=f"r{u}", name=f"r_{u}")
             for u in range(NUNITS)]

        def x_dma(u, chunk):
            sl = (2 * chunk) % NSLOT
            nc.sync.dma_start(out=R[u][64:128, :, sl:sl + 2, :],
                              in_=xab[chunk, :, u])

        for u in range(NUNITS):
            x_dma(u, 0)

        CP = [None] * (NUNITS // 2)   # c state per unit pair [128, 2, NB]
        GS_prev = None
        CP_pending = None

        for t in range(T):
            last = t == T - 1
            sl = t % NSLOT
            for u in range(NUNITS):
                p = u // 2
                ru = R[u]
                ps = ppool.tile([128, 4, NB], F32, tag="g", name=f"ps_{t}_{u}")
                for s in range(2):
                    nc.tensor.matmul(ps[:, s], BW[:, s, :], ones_sb,
                                     start=True, stop=False,
                                     skip_group_check=True)
                nc.vector.tensor_copy(out=ps[:, 2:4, :], in_=BI)
                for s in range(4):
                    lst = s == 3
                    if t == 0:
                        # h == 0: contract x only (K=64)
                        nc.tensor.matmul(ps[0:64, s], W[64:128, s, :],
                                         ru[64:128, 0, 0, :], start=False,
                                         stop=False, skip_group_check=True)
                        nc.tensor.matmul(ps[64:128, s], W[64:128, s, :],
                                         ru[64:128, 1, 0, :], start=False,
                                         stop=lst, skip_group_check=True)
                    else:
                        nc.tensor.matmul(ps[0:64, s], W[:, s, :],
                                         ru[:, 0, sl, :], start=False,
                                         stop=False, skip_group_check=True)
                        nc.tensor.matmul(ps[64:128, s], W[:, s, :],
                                         ru[:, 1, sl, :], start=False,
                                         stop=lst, skip_group_check=True)

                GS = gpool.tile([128, 4, NB], BF, tag="GS", name=f"gs_{t}_{u}")
                nc.scalar.activation(GS, ps, AF.Sigmoid)

                if u % 2 == 0:
                    CPnew = spool.tile([128, 2, NB], BF, tag=f"C{p}",
                                       name=f"c_{t}_{p}")
                    CP_pending = CPnew
                else:
                    CPnew = CP_pending
                if t == 0:
                    Gt = qpool.tile([128, NB], BF, tag="Gt", name=f"gt_{t}_{u}")
                    nc.vector.tensor_scalar(Gt, GS[:, 3], 2.0, -1.0,
                                            ALU.mult, ALU.add)
                    # c0 = i*g straight from the Pool multiply
                    nc.gpsimd.tensor_mul(CPnew[:, u % 2, :], GS[:, 0], Gt)
                else:
                    # ww only needs the sigmoid output: start it before Gt
                    ww = qpool.tile([128, NB], BF, tag="ww", name=f"ww_{t}_{u}")
                    nc.gpsimd.tensor_mul(ww, GS[:, 1], CP[p][:, u % 2, :])
                    Gt = qpool.tile([128, NB], BF, tag="Gt", name=f"gt_{t}_{u}")
                    # tanh(g) = 2*sigmoid(2g) - 1  (g weights/bias pre-doubled)
                    nc.vector.tensor_scalar(Gt, GS[:, 3], 2.0, -1.0,
                                            ALU.mult, ALU.add)
                    uu = qpool.tile([128, NB], BF, tag="uu", name=f"uu_{t}_{u}")
                    nc.gpsimd.tensor_mul(uu, GS[:, 0], Gt)
                    nc.vector.tensor_add(CPnew[:, u % 2, :], uu, ww)

                if u % 2 == 0:
                    GS_prev = GS
                    continue

                CP[p] = CP_pending
                # pair complete: tanh + h for both units of the pair
                TT = wpool.tile([128, 2, NB], BF, tag="TT", name=f"tt_{t}_{u}")
                nc.scalar.activation(TT, CP[p], AF.Tanh)
                for v in (u - 1, u):
                    GSv = GS_prev if v == u - 1 else GS
                    if not last:
                        if t % 2 == 0 and t + 2 < T:
                            x_dma(v, (t + 2) // 2)
                        sln = (t + 1) % NSLOT
                        rv = R[v]
                        nc.vector.tensor_mul(rv[0:64, 0, sln, :],
                                             GSv[0:64, 2], TT[0:64, v % 2])
                        nc.vector.tensor_mul(rv[0:64, 1, sln, :],
                                             GSv[64:128, 2], TT[64:128, v % 2])
                    else:
                        H = wpool.tile([64, 2, NB], BF, tag="H", name=f"h_{v}")
                        nc.vector.tensor_mul(H[:, 0, :], GSv[0:64, 2],
                                             TT[0:64, v % 2])
                        nc.vector.tensor_mul(H[:, 1, :], GSv[64:128, 2],
                                             TT[64:128, v % 2])
                        ob = qpool.tile([1, 2, NB], BF, tag="ob",
                                        name=f"ob_{v}")
                        for g in range(2):
                            op = ppool.tile([1, NB], F32, tag="g",
                                            name=f"op_{v}_{g}")
                            nc.tensor.matmul(op, BO, ones_sb,
                                             start=True, stop=False,
                                             skip_group_check=True)
                            nc.tensor.matmul(op, WO, H[:, g, :],
                                             start=False, stop=True,
                                             skip_group_check=True)
                            nc.vector.tensor_copy(out=ob[:, g, :], in_=op)
                        nc.sync.dma_start(out=y[v], in_=ob)


def prep_weights(W_ih, W_hh, b_ih, b_hh, W_out, b_out):
    """Host-side packing (numpy). Returns DRAM arrays for the kernel."""
    bf16 = ml_dtypes.bfloat16
    W = np.zeros((128, 4, 64), np.float32)      # rhs layout [h; x]
    BW = np.zeros((1, 4, 128), np.float32)
    b = (b_ih + b_hh).astype(np.float32)
    for s, gi in enumerate(SLICE_TO_TORCH_GATE):
        blk_ih = W_ih[gi * 64:(gi + 1) * 64, :].astype(np.float32)
        blk_hh = W_hh[gi * 64:(gi + 1) * 64, :].astype(np.float32)
        scale = 2.0 if s == 3 else 1.0
        W[0:64, s, :] = blk_hh.T * scale
        W[64:128, s, :] = blk_ih.T * scale
        bb = b[gi * 64:(gi + 1) * 64] * scale
        BW[0, s, 0:64] = bb
        BW[0, s, 64:128] = bb
    # bias image for banks 2 (o) and 3 (g)
    BI = np.stack([np.broadcast_to(BW[0, 2, :, None], (128, NB)),
                   np.broadcast_to(BW[0, 3, :, None], (128, NB))], axis=1)
    WO = W_out[0].astype(np.float32).reshape(64, 1)
    BO = np.full((1, 1), np.float32(b_out[0]))
    return {
        "Wd": W.astype(bf16),
        "BWd": BW.astype(bf16),
        "BId": np.ascontiguousarray(BI).astype(bf16),
        "WOd": WO.astype(bf16),
        "BOd": BO.astype(bf16),
    }


_BUILD_CACHE = {}


def build_nc():
    key = "nc_v6"
    if key in _BUILD_CACHE:
        return _BUILD_CACHE[key]
    nc = bacc.Bacc("TRN2", target_bir_lowering=False, debug=False)
    aps = {
        "xab": nc.dram_tensor("xab", [T // 2, INP, NUNITS, 2, 2, NB], BF,
                              kind="ExternalInput").ap(),
        "Wd": nc.dram_tensor("Wd", [128, 4, 64], BF,
                             kind="ExternalInput").ap(),
        "BWd": nc.dram_tensor("BWd", [1, 4, 128], BF, kind="ExternalInput").ap(),
        "BId": nc.dram_tensor("BId", [128, 2, NB], BF,
                              kind="ExternalInput").ap(),
        "WOd": nc.dram_tensor("WOd", [64, 1], BF, kind="ExternalInput").ap(),
        "BOd": nc.dram_tensor("BOd", [1, 1], BF, kind="ExternalInput").ap(),
        "y": nc.dram_tensor("y", [NUNITS, 1, 2, NB], BF,
                            kind="ExternalOutput").ap(),
    }
    with tile.TileContext(nc) as tc:
        emit_lstm(tc, aps)
    nc.compile()
    _BUILD_CACHE[key] = nc
    return nc


def make_in_maps(x, W_ih, W_hh, b_ih, b_hh, W_out, b_out):
    bf16 = ml_dtypes.bfloat16
    wd = prep_weights(W_ih, W_hh, b_ih, b_hh, W_out, b_out)
    xt = np.ascontiguousarray(x.transpose(1, 2, 0))   # [T, I, B] f32
    in_maps = []
    for c in range(NCORES):
        sl = xt[:, :, c * B_LOC:(c + 1) * B_LOC]
        # [T, I, B_loc] -> [T/2(chunk), 2(step), I, NU, 2(grp), NB]
        blk = sl.reshape(T // 2, 2, INP, NUNITS, 2, NB)
        xab = np.ascontiguousarray(
            blk.transpose(0, 2, 3, 4, 1, 5)).astype(bf16)
        in_maps.append({"xab": xab, **wd})
    return in_maps


def kernel(x, W_ih, W_hh, b_ih, b_hh, W_out, b_out):
    from concourse.bass_utils import run_bass_kernel_spmd

    nc = build_nc()
    in_maps = make_in_maps(x, W_ih, W_hh, b_ih, b_hh, W_out, b_out)
    res = run_bass_kernel_spmd(nc, in_maps, core_ids=list(range(NCORES)))
    y = np.concatenate([res.results[c]["y"].astype(np.float32).reshape(B_LOC)
                        for c in range(NCORES)])
    return y.reshape(B, 1).astype(np.float32)


# revision 14
# speedup vs baseline: 1.4221x; 1.4221x over previous
"""LSTM (B=131072, T=10, INP=HID=64) + linear head, data-parallel on 8 TRN2 cores.

v7 layout (per core, B_loc=16384, 16 "units" of two 512-col groups A/B):
  - Feature-major: features on SBUF partitions, batch on the free dim. PSUM
    per unit-step: [128, 4, NB] banks (i, f, g, o), bank = [gate_A(0:64);
    gate_B(64:128)], so all elementwise ops run 128 lanes wide.
  - Both groups use rhs layout [h(0:64); x(64:128)] inside one persistent
    tile RAB[128, group, slot(4), NB] per unit, sharing a single weight copy.
    h_A lands aligned; h_B is written with a cross-partition output (legal:
    only tensor-op *inputs* must share a base partition). x is DMA'd two
    steps ahead into slot pairs, so DMA WAR waits are against long-retired
    readers and the SP queue never head-of-line blocks.
  - Bias: banks i, f seeded by K=1 matmuls on PE; bank g by a DVE copy from
    an SBUF image; the o-gate sigmoid is a separate ACT instruction carrying
    its bias as a per-partition vector (free). g weights/bias pre-doubled so
    tanh(g) = 2*sig(2g)-1. tanh(c) batched across unit pairs.
  - Two-phase software pipeline with a LAG-pair lead: phase 1 (matmuls,
    sigmoids, gate products, c update) runs ahead; phase 2 (tanh, h-muls,
    head) for pair q is emitted LAG pairs later, so the ACT queue never
    stalls on the cross-engine elementwise chain.
  - Elementwise: i*g and f*c products on Pool(GPSIMD); Gt fix, g-bank seed,
    c-add, h-muls, head staging on DVE.
"""

import numpy as np
import ml_dtypes

import concourse.bass as bass
import concourse.mybir as mybir
from concourse import bacc
import concourse.tile as tile

HID = 64
INP = 64
T = 10
B = 131072
NCORES = 8
B_LOC = B // NCORES   # 16384
NB = 512              # batch columns per group
NUNITS = B_LOC // (2 * NB)  # 16 units of (A, B) groups
NSLOT = 4             # rhs time slots (2-step DMA chunks, 2-step prefetch)
LAG = 2               # pairs of phase-1 lead over phase 2

BF = mybir.dt.bfloat16
F32 = mybir.dt.float32
AF = mybir.ActivationFunctionType
ALU = mybir.AluOpType

# psum gate-slice order matches torch block order: 0=i, 1=f, 2=g, 3=o
GATE_SCALE = [1.0, 1.0, 2.0, 1.0]


def emit_lstm(tc, aps):
    nc = tc.nc
    xab, Wd, BWd, BIgd, BOgd, WOd, BOd, y = (
        aps["xab"], aps["Wd"], aps["BWd"], aps["BIgd"], aps["BOgd"],
        aps["WOd"], aps["BOd"], aps["y"])

    with (
        tc.tile_pool(name="const", bufs=1) as cpool,
        tc.tile_pool(name="rhs", bufs=1) as rpool,
        tc.tile_pool(name="cstate", bufs=2) as spool,
        tc.tile_pool(name="gs", bufs=4) as gpool,
        tc.tile_pool(name="og", bufs=8) as opool,
        tc.tile_pool(name="work", bufs=2) as wpool,
        tc.tile_pool(name="small", bufs=3) as qpool,
        tc.tile_pool(name="psum", bufs=2, space="PSUM") as ppool,
    ):
        W = cpool.tile([128, 4, 64], BF)    # [k(h;x), slice, m]
        nc.sync.dma_start(out=W, in_=Wd)
        BW = cpool.tile([1, 2, 128], BF)    # i, f seed rows
        nc.sync.dma_start(out=BW, in_=BWd)
        BIg = cpool.tile([128, NB], BF)     # g-bank bias image
        nc.sync.dma_start(out=BIg, in_=BIgd)
        BOg = cpool.tile([128, 1], BF)      # o-gate ACT bias vector
        nc.sync.dma_start(out=BOg, in_=BOgd)
        WO = cpool.tile([64, 1], BF)
        nc.sync.dma_start(out=WO, in_=WOd)
        BO = cpool.tile([1, 1], BF)
        nc.sync.dma_start(out=BO, in_=BOd)
        ones_sb = cpool.tile([1, NB], BF)
        nc.vector.memset(ones_sb, 1.0)

        # persistent rhs tiles: [h(0:64); x(64:128)] x group x slot
        R = [rpool.tile([128, 2, NSLOT, NB], BF, tag=f"r{u}", name=f"r_{u}")
             for u in range(NUNITS)]

        def x_dma(u, chunk):
            sl = (2 * chunk) % NSLOT
            nc.sync.dma_start(out=R[u][64:128, :, sl:sl + 2, :],
                              in_=xab[chunk, :, u])

        for u in range(NUNITS):
            x_dma(u, 0)

        CP = [(None, None)] * (NUNITS // 2)  # (new, old) c tiles per pair
        OO = {}                              # (t, u) -> o-gate sigmoid tile

        def phase1(t, u):
            p = u // 2
            sl = t % NSLOT
            ru = R[u]
            ps = ppool.tile([128, 4, NB], F32, tag="g", name=f"ps_{t}_{u}")
            for s in range(2):
                nc.tensor.matmul(ps[:, s], BW[:, s, :], ones_sb,
                                 start=True, stop=False,
                                 skip_group_check=True)
            nc.vector.tensor_copy(out=ps[:, 2, :], in_=BIg)
            for s in range(4):
                lst = s == 3
                st = s == 3   # o-bank has no seed: first matmul clears psum
                if t == 0:
                    nc.tensor.matmul(ps[0:64, s], W[64:128, s, :],
                                     ru[64:128, 0, 0, :], start=st,
                                     stop=False, skip_group_check=True)
                    nc.tensor.matmul(ps[64:128, s], W[64:128, s, :],
                                     ru[64:128, 1, 0, :], start=st,
                                     stop=lst, skip_group_check=True)
                else:
                    nc.tensor.matmul(ps[0:64, s], W[:, s, :],
                                     ru[:, 0, sl, :], start=st,
                                     stop=False, skip_group_check=True)
                    nc.tensor.matmul(ps[64:128, s], W[:, s, :],
                                     ru[:, 1, sl, :], start=st,
                                     stop=lst, skip_group_check=True)

            GS = gpool.tile([128, 3, NB], BF, tag="GS", name=f"gs_{t}_{u}")
            nc.scalar.activation(GS, ps[:, 0:3, :], AF.Sigmoid)
            O = opool.tile([128, NB], BF, tag="O", name=f"o_{t}_{u}")
            nc.scalar.activation(O, ps[:, 3, :], AF.Sigmoid, bias=BOg)
            OO[(t, u)] = O

            if u % 2 == 0:
                CPn = spool.tile([128, 2, NB], BF, tag=f"C{p}",
                                 name=f"c_{t}_{p}")
                CP[p] = (CPn, CP[p][0])
            CPnew, CPold = CP[p]
            if t == 0:
                Gt = qpool.tile([128, NB], BF, tag="Gt", name=f"gt_{t}_{u}")
                nc.vector.tensor_scalar(Gt, GS[:, 2], 2.0, -1.0,
                                        ALU.mult, ALU.add)
                nc.gpsimd.tensor_mul(CPnew[:, u % 2, :], GS[:, 0], Gt)
            else:
                # f*c only needs the sigmoid output: start it before Gt
                ww = qpool.tile([128, NB], BF, tag="ww", name=f"ww_{t}_{u}")
                nc.gpsimd.tensor_mul(ww, GS[:, 1], CPold[:, u % 2, :])
                Gt = qpool.tile([128, NB], BF, tag="Gt", name=f"gt_{t}_{u}")
                # tanh(g) = 2*sigmoid(2g) - 1  (g weights/bias pre-doubled)
                nc.vector.tensor_scalar(Gt, GS[:, 2], 2.0, -1.0,
                                        ALU.mult, ALU.add)
                uu = qpool.tile([128, NB], BF, tag="uu", name=f"uu_{t}_{u}")
                nc.gpsimd.tensor_mul(uu, GS[:, 0], Gt)
                nc.vector.tensor_add(CPnew[:, u % 2, :], uu, ww)

        def phase2(t, q):
            last = t == T - 1
            TT = wpool.tile([128, 2, NB], BF, tag="TT", name=f"tt_{t}_{q}")
            nc.scalar.activation(TT, CP[q][0], AF.Tanh)
            for v in (2 * q, 2 * q + 1):
                O = OO.pop((t, v))
                if not last:
                    if t % 2 == 0 and t + 2 < T:
                        x_dma(v, (t + 2) // 2)
                    sln = (t + 1) % NSLOT
                    rv = R[v]
                    nc.vector.tensor_mul(rv[0:64, 0, sln, :],
                                         O[0:64, :], TT[0:64, v % 2])
                    nc.vector.tensor_mul(rv[0:64, 1, sln, :],
                                         O[64:128, :], TT[64:128, v % 2])
                else:
                    H = wpool.tile([64, 2, NB], BF, tag="H", name=f"h_{v}")
                    nc.vector.tensor_mul(H[:, 0, :], O[0:64, :],
                                         TT[0:64, v % 2])
                    nc.vector.tensor_mul(H[:, 1, :], O[64:128, :],
                                         TT[64:128, v % 2])
                    ob = qpool.tile([1, 2, NB], BF, tag="ob", name=f"ob_{v}")
                    for g in range(2):
                        op = ppool.tile([1, NB], F32, tag="g",
                                        name=f"op_{v}_{g}")
                        nc.tensor.matmul(op, BO, ones_sb,
                                         start=True, stop=False,
                                         skip_group_check=True)
                        nc.tensor.matmul(op, WO, H[:, g, :],
                                         start=False, stop=True,
                                         skip_group_check=True)
                        nc.vector.tensor_copy(out=ob[:, g, :], in_=op)
                    nc.sync.dma_start(out=y[v], in_=ob)

        pending = []
        for t in range(T):
            for u in range(NUNITS):
                phase1(t, u)
                if u % 2 == 1:
                    pending.append((t, u // 2))
                    if len(pending) > LAG:
                        phase2(*pending.pop(0))
        while pending:
            phase2(*pending.pop(0))


def prep_weights(W_ih, W_hh, b_ih, b_hh, W_out, b_out):
    """Host-side packing (numpy). Returns DRAM arrays for the kernel."""
    bf16 = ml_dtypes.bfloat16
    W = np.zeros((128, 4, 64), np.float32)      # rhs layout [h; x]
    BIAS = np.zeros((4, 128), np.float32)
    b = (b_ih + b_hh).astype(np.float32)
    for s in range(4):
        blk_ih = W_ih[s * 64:(s + 1) * 64, :].astype(np.float32)
        blk_hh = W_hh[s * 64:(s + 1) * 64, :].astype(np.float32)
        scale = GATE_SCALE[s]
        W[0:64, s, :] = blk_hh.T * scale
        W[64:128, s, :] = blk_ih.T * scale
        bb = b[s * 64:(s + 1) * 64] * scale
        BIAS[s, 0:64] = bb
        BIAS[s, 64:128] = bb
    BW = BIAS[0:2][None, :, :]                   # [1, 2, 128] i, f seeds
    BIg = np.broadcast_to(BIAS[2][:, None], (128, NB))
    BOg = BIAS[3][:, None]                       # [128, 1] o-gate ACT bias
    WO = W_out[0].astype(np.float32).reshape(64, 1)
    BO = np.full((1, 1), np.float32(b_out[0]))
    return {
        "Wd": W.astype(bf16),
        "BWd": np.ascontiguousarray(BW).astype(bf16),
        "BIgd": np.ascontiguousarray(BIg).astype(bf16),
        "BOgd": np.ascontiguousarray(BOg).astype(bf16),
        "WOd": WO.astype(bf16),
        "BOd": BO.astype(bf16),
    }


_BUILD_CACHE = {}


def build_nc():
    key = "nc_v7"
    if key in _BUILD_CACHE:
        return _BUILD_CACHE[key]
    nc = bacc.Bacc("TRN2", target_bir_lowering=False, debug=False)
    aps = {
        "xab": nc.dram_tensor("xab", [T // 2, INP, NUNITS, 2, 2, NB], BF,
                              kind="ExternalInput").ap(),
        "Wd": nc.dram_tensor("Wd", [128, 4, 64], BF,
                             kind="ExternalInput").ap(),
        "BWd": nc.dram_tensor("BWd", [1, 2, 128], BF, kind="ExternalInput").ap(),
        "BIgd": nc.dram_tensor("BIgd", [128, NB], BF,
                               kind="ExternalInput").ap(),
        "BOgd": nc.dram_tensor("BOgd", [128, 1], BF,
                               kind="ExternalInput").ap(),
        "WOd": nc.dram_tensor("WOd", [64, 1], BF, kind="ExternalInput").ap(),
        "BOd": nc.dram_tensor("BOd", [1, 1], BF, kind="ExternalInput").ap(),
        "y": nc.dram_tensor("y", [NUNITS, 1, 2, NB], BF,
                            kind="ExternalOutput").ap(),
    }
    with tile.TileContext(nc) as tc:
        emit_lstm(tc, aps)
    nc.compile()
    _BUILD_CACHE[key] = nc
    return nc


def make_in_maps(x, W_ih, W_hh, b_ih, b_hh, W_out, b_out):
    bf16 = ml_dtypes.bfloat16
    wd = prep_weights(W_ih, W_hh, b_ih, b_hh, W_out, b_out)
    xt = np.ascontiguousarray(x.transpose(1, 2, 0))   # [T, I, B] f32
    in_maps = []
    for c in range(NCORES):
        sl = xt[:, :, c * B_LOC:(c + 1) * B_LOC]
        # [T, I, B_loc] -> [T/2(chunk), 2(step), I, NU, 2(grp), NB]
        blk = sl.reshape(T // 2, 2, INP, NUNITS, 2, NB)
        xab = np.ascontiguousarray(
            blk.transpose(0, 2, 3, 4, 1, 5)).astype(bf16)
        in_maps.append({"xab": xab, **wd})
    return in_maps


def kernel(x, W_ih, W_hh, b_ih, b_hh, W_out, b_out):
    from concourse.bass_utils import run_bass_kernel_spmd

    nc = build_nc()
    in_maps = make_in_maps(x, W_ih, W_hh, b_ih, b_hh, W_out, b_out)
    res = run_bass_kernel_spmd(nc, in_maps, core_ids=list(range(NCORES)))
    y = np.concatenate([res.results[c]["y"].astype(np.float32).reshape(B_LOC)
                        for c in range(NCORES)])
    return y.reshape(B, 1).astype(np.float32)


# revision 15
# speedup vs baseline: 1.4531x; 1.0218x over previous
"""LSTM (B=131072, T=10, INP=HID=64) + linear head, data-parallel on 8 TRN2 cores.

v7 layout (per core, B_loc=16384, 16 "units" of two 512-col groups A/B):
  - Feature-major: features on SBUF partitions, batch on the free dim. PSUM
    per unit-step: [128, 4, NB] banks (i, f, g, o), bank = [gate_A(0:64);
    gate_B(64:128)], so all elementwise ops run 128 lanes wide.
  - Both groups use rhs layout [h(0:64); x(64:128)] inside one persistent
    tile RAB[128, group, slot(4), NB] per unit, sharing a single weight copy.
    h_A lands aligned; h_B is written with a cross-partition output (legal:
    only tensor-op *inputs* must share a base partition). x is DMA'd two
    steps ahead into slot pairs, so DMA WAR waits are against long-retired
    readers and the SP queue never head-of-line blocks.
  - Bias: banks i, f seeded by K=1 matmuls on PE; bank g by a DVE copy from
    an SBUF image; the o-gate sigmoid is a separate ACT instruction carrying
    its bias as a per-partition vector (free). g weights/bias pre-doubled so
    tanh(g) = 2*sig(2g)-1. tanh(c) batched across unit pairs.
  - Two-phase software pipeline with a LAG-pair lead: phase 1 (matmuls,
    sigmoids, gate products, c update) runs ahead; phase 2 (tanh, h-muls,
    head) for pair q is emitted LAG pairs later, so the ACT queue never
    stalls on the cross-engine elementwise chain.
  - Elementwise: i*g and f*c products on Pool(GPSIMD); Gt fix, g-bank seed,
    c-add, h-muls, head staging on DVE.
"""

import numpy as np
import ml_dtypes

import concourse.bass as bass
import concourse.mybir as mybir
from concourse import bacc
import concourse.tile as tile

HID = 64
INP = 64
T = 10
B = 131072
NCORES = 8
B_LOC = B // NCORES   # 16384
NB = 512              # batch columns per group
NUNITS = B_LOC // (2 * NB)  # 16 units of (A, B) groups
NSLOT = 4             # rhs time slots (2-step DMA chunks, 2-step prefetch)
LAG = 2               # pairs of phase-1 lead over phase 2

BF = mybir.dt.bfloat16
F32 = mybir.dt.float32
AF = mybir.ActivationFunctionType
ALU = mybir.AluOpType

# psum gate-slice order matches torch block order: 0=i, 1=f, 2=g, 3=o
GATE_SCALE = [1.0, 1.0, 2.0, 1.0]


def emit_lstm(tc, aps):
    nc = tc.nc
    xab, Wd, BWd, BIgd, WOd, BOd, y = (
        aps["xab"], aps["Wd"], aps["BWd"], aps["BIgd"],
        aps["WOd"], aps["BOd"], aps["y"])

    with (
        tc.tile_pool(name="const", bufs=1) as cpool,
        tc.tile_pool(name="rhs", bufs=1) as rpool,
        tc.tile_pool(name="cstate", bufs=2) as spool,
        tc.tile_pool(name="gs", bufs=6) as gpool,
        tc.tile_pool(name="work", bufs=2) as wpool,
        tc.tile_pool(name="small", bufs=3) as qpool,
        tc.tile_pool(name="psum", bufs=2, space="PSUM") as ppool,
    ):
        W = cpool.tile([128, 4, 64], BF)    # [k(h;x), slice, m]
        nc.sync.dma_start(out=W, in_=Wd)
        BW = cpool.tile([1, 2, 128], BF)    # i, f seed rows
        nc.sync.dma_start(out=BW, in_=BWd)
        BIg = cpool.tile([128, 2, NB], BF)  # g+o bank bias image
        nc.sync.dma_start(out=BIg, in_=BIgd)
        WO = cpool.tile([64, 1], BF)
        nc.sync.dma_start(out=WO, in_=WOd)
        BO = cpool.tile([1, 1], BF)
        nc.sync.dma_start(out=BO, in_=BOd)
        ones_sb = cpool.tile([1, NB], BF)
        nc.vector.memset(ones_sb, 1.0)

        # persistent rhs tiles: [h(0:64); x(64:128)] x group x slot
        R = [rpool.tile([128, 2, NSLOT, NB], BF, tag=f"r{u}", name=f"r_{u}")
             for u in range(NUNITS)]

        def x_dma(u, chunk):
            sl = (2 * chunk) % NSLOT
            nc.sync.dma_start(out=R[u][64:128, :, sl:sl + 2, :],
                              in_=xab[chunk, :, u])

        for u in range(NUNITS):
            x_dma(u, 0)

        CP = [(None, None)] * (NUNITS // 2)  # (new, old) c tiles per pair
        OO = {}                              # (t, u) -> o-gate sigmoid tile

        def phase1(t, u):
            p = u // 2
            sl = t % NSLOT
            ru = R[u]
            ps = ppool.tile([128, 4, NB], F32, tag="g", name=f"ps_{t}_{u}")
            for s in range(2):
                nc.tensor.matmul(ps[:, s], BW[:, s, :], ones_sb,
                                 start=True, stop=False,
                                 skip_group_check=True)
            nc.vector.tensor_copy(out=ps[:, 2:4, :], in_=BIg)
            for s in range(4):
                lst = s == 3
                st = False
                if t == 0:
                    nc.tensor.matmul(ps[0:64, s], W[64:128, s, :],
                                     ru[64:128, 0, 0, :], start=st,
                                     stop=False, skip_group_check=True)
                    nc.tensor.matmul(ps[64:128, s], W[64:128, s, :],
                                     ru[64:128, 1, 0, :], start=st,
                                     stop=lst, skip_group_check=True)
                else:
                    nc.tensor.matmul(ps[0:64, s], W[:, s, :],
                                     ru[:, 0, sl, :], start=st,
                                     stop=False, skip_group_check=True)
                    nc.tensor.matmul(ps[64:128, s], W[:, s, :],
                                     ru[:, 1, sl, :], start=st,
                                     stop=lst, skip_group_check=True)

            GS = gpool.tile([128, 4, NB], BF, tag="GS", name=f"gs_{t}_{u}")
            nc.scalar.activation(GS, ps, AF.Sigmoid)
            OO[(t, u)] = GS

            if u % 2 == 0:
                CPn = spool.tile([128, 2, NB], BF, tag=f"C{p}",
                                 name=f"c_{t}_{p}")
                CP[p] = (CPn, CP[p][0])
            CPnew, CPold = CP[p]
            Gt = qpool.tile([128, NB], BF, tag="Gt", name=f"gt_{t}_{u}")
            nc.gpsimd.tensor_scalar(Gt, GS[:, 2], 2.0, -1.0,
                                    ALU.mult, ALU.add)
            if t == 0:
                nc.gpsimd.tensor_mul(CPnew[:, u % 2, :], GS[:, 0], Gt)
            else:
                ww = qpool.tile([128, NB], BF, tag="ww", name=f"ww_{t}_{u}")
                nc.vector.tensor_mul(ww, GS[:, 1], CPold[:, u % 2, :])
                uu = qpool.tile([128, NB], BF, tag="uu", name=f"uu_{t}_{u}")
                nc.gpsimd.tensor_mul(uu, GS[:, 0], Gt)
                nc.vector.tensor_add(CPnew[:, u % 2, :], uu, ww)

        def phase2(t, q):
            last = t == T - 1
            TT = wpool.tile([128, 2, NB], BF, tag="TT", name=f"tt_{t}_{q}")
            nc.scalar.activation(TT, CP[q][0], AF.Tanh)
            for v in (2 * q, 2 * q + 1):
                O = OO.pop((t, v))[:, 3]
                if not last:
                    if t % 2 == 0 and t + 2 < T:
                        x_dma(v, (t + 2) // 2)
                    sln = (t + 1) % NSLOT
                    rv = R[v]
                    nc.vector.tensor_mul(rv[0:64, 0, sln, :],
                                         O[0:64, :], TT[0:64, v % 2])
                    nc.vector.tensor_mul(rv[0:64, 1, sln, :],
                                         O[64:128, :], TT[64:128, v % 2])
                else:
                    H = wpool.tile([64, 2, NB], BF, tag="TT", name=f"h_{v}")
                    nc.vector.tensor_mul(H[:, 0, :], O[0:64, :],
                                         TT[0:64, v % 2])
                    nc.vector.tensor_mul(H[:, 1, :], O[64:128, :],
                                         TT[64:128, v % 2])
                    ob = qpool.tile([1, 2, NB], BF, tag="ob", name=f"ob_{v}")
                    for g in range(2):
                        op = ppool.tile([1, NB], F32, tag="g",
                                        name=f"op_{v}_{g}")
                        nc.tensor.matmul(op, BO, ones_sb,
                                         start=True, stop=False,
                                         skip_group_check=True)
                        nc.tensor.matmul(op, WO, H[:, g, :],
                                         start=False, stop=True,
                                         skip_group_check=True)
                        nc.vector.tensor_copy(out=ob[:, g, :], in_=op)
                    nc.sync.dma_start(out=y[v], in_=ob)

        pending = []
        for t in range(T):
            for u in range(NUNITS):
                phase1(t, u)
                if u % 2 == 1:
                    pending.append((t, u // 2))
                    if len(pending) > LAG:
                        phase2(*pending.pop(0))
        while pending:
            phase2(*pending.pop(0))


def prep_weights(W_ih, W_hh, b_ih, b_hh, W_out, b_out):
    """Host-side packing (numpy). Returns DRAM arrays for the kernel."""
    bf16 = ml_dtypes.bfloat16
    W = np.zeros((128, 4, 64), np.float32)      # rhs layout [h; x]
    BIAS = np.zeros((4, 128), np.float32)
    b = (b_ih + b_hh).astype(np.float32)
    for s in range(4):
        blk_ih = W_ih[s * 64:(s + 1) * 64, :].astype(np.float32)
        blk_hh = W_hh[s * 64:(s + 1) * 64, :].astype(np.float32)
        scale = GATE_SCALE[s]
        W[0:64, s, :] = blk_hh.T * scale
        W[64:128, s, :] = blk_ih.T * scale
        bb = b[s * 64:(s + 1) * 64] * scale
        BIAS[s, 0:64] = bb
        BIAS[s, 64:128] = bb
    BW = BIAS[0:2][None, :, :]                   # [1, 2, 128] i, f seeds
    BIg = np.stack([np.broadcast_to(BIAS[2][:, None], (128, NB)),
                    np.broadcast_to(BIAS[3][:, None], (128, NB))], axis=1)
    WO = W_out[0].astype(np.float32).reshape(64, 1)
    BO = np.full((1, 1), np.float32(b_out[0]))
    return {
        "Wd": W.astype(bf16),
        "BWd": np.ascontiguousarray(BW).astype(bf16),
        "BIgd": np.ascontiguousarray(BIg).astype(bf16),
        "WOd": WO.astype(bf16),
        "BOd": BO.astype(bf16),
    }


_BUILD_CACHE = {}


def build_nc():
    key = "nc_v7"
    if key in _BUILD_CACHE:
        return _BUILD_CACHE[key]
    nc = bacc.Bacc("TRN2", target_bir_lowering=False, debug=False)
    aps = {
        "xab": nc.dram_tensor("xab", [T // 2, INP, NUNITS, 2, 2, NB], BF,
                              kind="ExternalInput").ap(),
        "Wd": nc.dram_tensor("Wd", [128, 4, 64], BF,
                             kind="ExternalInput").ap(),
        "BWd": nc.dram_tensor("BWd", [1, 2, 128], BF, kind="ExternalInput").ap(),
        "BIgd": nc.dram_tensor("BIgd", [128, 2, NB], BF,
                               kind="ExternalInput").ap(),
        "WOd": nc.dram_tensor("WOd", [64, 1], BF, kind="ExternalInput").ap(),
        "BOd": nc.dram_tensor("BOd", [1, 1], BF, kind="ExternalInput").ap(),
        "y": nc.dram_tensor("y", [NUNITS, 1, 2, NB], BF,
                            kind="ExternalOutput").ap(),
    }
    with tile.TileContext(nc) as tc:
        emit_lstm(tc, aps)
    nc.compile()
    _BUILD_CACHE[key] = nc
    return nc


def make_in_maps(x, W_ih, W_hh, b_ih, b_hh, W_out, b_out):
    bf16 = ml_dtypes.bfloat16
    wd = prep_weights(W_ih, W_hh, b_ih, b_hh, W_out, b_out)
    xt = np.ascontiguousarray(x.transpose(1, 2, 0))   # [T, I, B] f32
    in_maps = []
    for c in range(NCORES):
        sl = xt[:, :, c * B_LOC:(c + 1) * B_LOC]
        # [T, I, B_loc] -> [T/2(chunk), 2(step), I, NU, 2(grp), NB]
        blk = sl.reshape(T // 2, 2, INP, NUNITS, 2, NB)
        xab = np.ascontiguousarray(
            blk.transpose(0, 2, 3, 4, 1, 5)).astype(bf16)
        in_maps.append({"xab": xab, **wd})
    return in_maps


def kernel(x, W_ih, W_hh, b_ih, b_hh, W_out, b_out):
    from concourse.bass_utils import run_bass_kernel_spmd

    nc = build_nc()
    in_maps = make_in_maps(x, W_ih, W_hh, b_ih, b_hh, W_out, b_out)
    res = run_bass_kernel_spmd(nc, in_maps, core_ids=list(range(NCORES)))
    y = np.concatenate([res.results[c]["y"].astype(np.float32).reshape(B_LOC)
                        for c in range(NCORES)])
    return y.reshape(B, 1).astype(np.float32)


# revision 16
# speedup vs baseline: 1.4910x; 1.0261x over previous
"""LSTM (B=131072, T=10, INP=HID=64) + linear head, data-parallel on 8 TRN2 cores.

v7 layout (per core, B_loc=16384, 16 "units" of two 512-col groups A/B):
  - Feature-major: features on SBUF partitions, batch on the free dim. PSUM
    per unit-step: [128, 4, NB] banks (i, f, g, o), bank = [gate_A(0:64);
    gate_B(64:128)], so all elementwise ops run 128 lanes wide.
  - Both groups use rhs layout [h(0:64); x(64:128)] inside one persistent
    tile RAB[128, group, slot(4), NB] per unit, sharing a single weight copy.
    h_A lands aligned; h_B is written with a cross-partition output (legal:
    only tensor-op *inputs* must share a base partition). x is DMA'd two
    steps ahead into slot pairs, so DMA WAR waits are against long-retired
    readers and the SP queue never head-of-line blocks.
  - Bias: banks i, f seeded by K=1 matmuls on PE; bank g by a DVE copy from
    an SBUF image; the o-gate sigmoid is a separate ACT instruction carrying
    its bias as a per-partition vector (free). g weights/bias pre-doubled so
    tanh(g) = 2*sig(2g)-1. tanh(c) batched across unit pairs.
  - Two-phase software pipeline with a LAG-pair lead: phase 1 (matmuls,
    sigmoids, gate products, c update) runs ahead; phase 2 (tanh, h-muls,
    head) for pair q is emitted LAG pairs later, so the ACT queue never
    stalls on the cross-engine elementwise chain.
  - Elementwise: i*g and f*c products on Pool(GPSIMD); Gt fix, g-bank seed,
    c-add, h-muls, head staging on DVE.
"""

import numpy as np
import ml_dtypes

import concourse.bass as bass
import concourse.mybir as mybir
from concourse import bacc
import concourse.tile as tile

HID = 64
INP = 64
T = 10
B = 131072
NCORES = 8
B_LOC = B // NCORES   # 16384
NB = 512              # batch columns per group
NUNITS = B_LOC // (2 * NB)  # 16 units of (A, B) groups
NSLOT = 4             # rhs time slots (2-step DMA chunks, 2-step prefetch)
LAG = 2               # pairs of phase-1 lead over phase 2

BF = mybir.dt.bfloat16
F32 = mybir.dt.float32
AF = mybir.ActivationFunctionType
ALU = mybir.AluOpType

# psum gate-slice order matches torch block order: 0=i, 1=f, 2=g, 3=o
GATE_SCALE = [1.0, 1.0, 2.0, 1.0]


def emit_lstm(tc, aps):
    nc = tc.nc
    xab, Wd, BWd, BIgd, WOd, BOd, y = (
        aps["xab"], aps["Wd"], aps["BWd"], aps["BIgd"],
        aps["WOd"], aps["BOd"], aps["y"])

    with (
        tc.tile_pool(name="const", bufs=1) as cpool,
        tc.tile_pool(name="rhs", bufs=1) as rpool,
        tc.tile_pool(name="cstate", bufs=2) as spool,
        tc.tile_pool(name="gs", bufs=6) as gpool,
        tc.tile_pool(name="work", bufs=4) as wpool,
        tc.tile_pool(name="small", bufs=3) as qpool,
        tc.tile_pool(name="psum", bufs=2, space="PSUM") as ppool,
    ):
        W = cpool.tile([128, 4, 64], BF)    # [k(h;x), slice, m]
        nc.sync.dma_start(out=W, in_=Wd)
        BW = cpool.tile([1, 3, 128], BF)    # i, f, g seed rows
        nc.sync.dma_start(out=BW, in_=BWd)
        BIg = cpool.tile([128, NB], BF)     # o bank bias image
        nc.sync.dma_start(out=BIg, in_=BIgd)
        WO = cpool.tile([64, 1], BF)
        nc.sync.dma_start(out=WO, in_=WOd)
        BO = cpool.tile([1, 1], BF)
        nc.sync.dma_start(out=BO, in_=BOd)
        ones_sb = cpool.tile([1, NB], BF)
        nc.vector.memset(ones_sb, 1.0)

        # persistent rhs tiles: [h(0:64); x(64:128)] x group x slot
        R = [rpool.tile([128, 2, NSLOT, NB], BF, tag=f"r{u}", name=f"r_{u}")
             for u in range(NUNITS)]

        def x_dma(u, chunk):
            sl = (2 * chunk) % NSLOT
            nc.sync.dma_start(out=R[u][64:128, :, sl:sl + 2, :],
                              in_=xab[chunk, :, u])

        for u in range(NUNITS):
            x_dma(u, 0)

        CP = [(None, None)] * (NUNITS // 2)  # (new, old) c tiles per pair
        OO = {}                              # (t, u) -> o-gate sigmoid tile

        def phase1(t, u):
            p = u // 2
            sl = t % NSLOT
            ru = R[u]
            ps = ppool.tile([128, 4, NB], F32, tag="g", name=f"ps_{t}_{u}")
            for s in range(3):
                nc.tensor.matmul(ps[:, s], BW[:, s, :], ones_sb,
                                 start=True, stop=False,
                                 skip_group_check=True)
            nc.vector.tensor_copy(out=ps[:, 3, :], in_=BIg)
            for s in range(4):
                lst = s == 3
                st = False
                if t == 0:
                    nc.tensor.matmul(ps[0:64, s], W[64:128, s, :],
                                     ru[64:128, 0, 0, :], start=st,
                                     stop=False, skip_group_check=True)
                    nc.tensor.matmul(ps[64:128, s], W[64:128, s, :],
                                     ru[64:128, 1, 0, :], start=st,
                                     stop=lst, skip_group_check=True)
                else:
                    nc.tensor.matmul(ps[0:64, s], W[:, s, :],
                                     ru[:, 0, sl, :], start=st,
                                     stop=False, skip_group_check=True)
                    nc.tensor.matmul(ps[64:128, s], W[:, s, :],
                                     ru[:, 1, sl, :], start=st,
                                     stop=lst, skip_group_check=True)

            GS = gpool.tile([128, 4, NB], BF, tag="GS", name=f"gs_{t}_{u}")
            nc.scalar.activation(GS, ps, AF.Sigmoid)
            OO[(t, u)] = GS

            if u % 2 == 0:
                CPn = spool.tile([128, 2, NB], BF, tag=f"C{p}",
                                 name=f"c_{t}_{p}")
                CP[p] = (CPn, CP[p][0])
            CPnew, CPold = CP[p]
            Gt = qpool.tile([128, NB], BF, tag="Gt", name=f"gt_{t}_{u}")
            nc.gpsimd.tensor_scalar(Gt, GS[:, 2], 2.0, -1.0,
                                    ALU.mult, ALU.add)
            if t == 0:
                nc.gpsimd.tensor_mul(CPnew[:, u % 2, :], GS[:, 0], Gt)
            else:
                ww = qpool.tile([128, NB], BF, tag="ww", name=f"ww_{t}_{u}")
                nc.vector.tensor_mul(ww, GS[:, 1], CPold[:, u % 2, :])
                uu = qpool.tile([128, NB], BF, tag="uu", name=f"uu_{t}_{u}")
                nc.gpsimd.tensor_mul(uu, GS[:, 0], Gt)
                nc.vector.tensor_add(CPnew[:, u % 2, :], uu, ww)

        def phase2(t, q):
            last = t == T - 1
            TT = wpool.tile([128, 2, NB], BF, tag="TT", name=f"tt_{t}_{q}")
            nc.scalar.activation(TT, CP[q][0], AF.Tanh)
            for v in (2 * q, 2 * q + 1):
                O = OO.pop((t, v))[:, 3]
                if not last:
                    if t % 2 == 0 and t + 2 < T:
                        x_dma(v, (t + 2) // 2)
                    sln = (t + 1) % NSLOT
                    rv = R[v]
                    nc.vector.tensor_mul(rv[0:64, 0, sln, :],
                                         O[0:64, :], TT[0:64, v % 2])
                    nc.vector.tensor_mul(rv[0:64, 1, sln, :],
                                         O[64:128, :], TT[64:128, v % 2])
                else:
                    H = wpool.tile([64, 2, NB], BF, tag="TT", name=f"h_{v}")
                    nc.vector.tensor_mul(H[:, 0, :], O[0:64, :],
                                         TT[0:64, v % 2])
                    nc.vector.tensor_mul(H[:, 1, :], O[64:128, :],
                                         TT[64:128, v % 2])
                    ob = wpool.tile([1, 2, NB], BF, tag="TT", name=f"ob_{v}")
                    for g in range(2):
                        op = ppool.tile([1, NB], F32, tag="g",
                                        name=f"op_{v}_{g}")
                        nc.tensor.matmul(op, BO, ones_sb,
                                         start=True, stop=False,
                                         skip_group_check=True)
                        nc.tensor.matmul(op, WO, H[:, g, :],
                                         start=False, stop=True,
                                         skip_group_check=True)
                        nc.vector.tensor_copy(out=ob[:, g, :], in_=op)
                    nc.sync.dma_start(out=y[v], in_=ob)

        pending = []
        for t in range(T):
            for u in range(NUNITS):
                phase1(t, u)
                if u % 2 == 1:
                    pending.append((t, u // 2))
                    if len(pending) > LAG:
                        phase2(*pending.pop(0))
        while pending:
            phase2(*pending.pop(0))


def prep_weights(W_ih, W_hh, b_ih, b_hh, W_out, b_out):
    """Host-side packing (numpy). Returns DRAM arrays for the kernel."""
    bf16 = ml_dtypes.bfloat16
    W = np.zeros((128, 4, 64), np.float32)      # rhs layout [h; x]
    BIAS = np.zeros((4, 128), np.float32)
    b = (b_ih + b_hh).astype(np.float32)
    for s in range(4):
        blk_ih = W_ih[s * 64:(s + 1) * 64, :].astype(np.float32)
        blk_hh = W_hh[s * 64:(s + 1) * 64, :].astype(np.float32)
        scale = GATE_SCALE[s]
        W[0:64, s, :] = blk_hh.T * scale
        W[64:128, s, :] = blk_ih.T * scale
        bb = b[s * 64:(s + 1) * 64] * scale
        BIAS[s, 0:64] = bb
        BIAS[s, 64:128] = bb
    BW = BIAS[0:3][None, :, :]                   # [1, 3, 128] i, f, g seeds
    BIg = np.broadcast_to(BIAS[3][:, None], (128, NB))
    WO = W_out[0].astype(np.float32).reshape(64, 1)
    BO = np.full((1, 1), np.float32(b_out[0]))
    return {
        "Wd": W.astype(bf16),
        "BWd": np.ascontiguousarray(BW).astype(bf16),
        "BIgd": np.ascontiguousarray(BIg).astype(bf16),
        "WOd": WO.astype(bf16),
        "BOd": BO.astype(bf16),
    }


_BUILD_CACHE = {}


def build_nc():
    key = "nc_v7"
    if key in _BUILD_CACHE:
        return _BUILD_CACHE[key]
    nc = bacc.Bacc("TRN2", target_bir_lowering=False, debug=False)
    aps = {
        "xab": nc.dram_tensor("xab", [T // 2, INP, NUNITS, 2, 2, NB], BF,
                              kind="ExternalInput").ap(),
        "Wd": nc.dram_tensor("Wd", [128, 4, 64], BF,
                             kind="ExternalInput").ap(),
        "BWd": nc.dram_tensor("BWd", [1, 3, 128], BF, kind="ExternalInput").ap(),
        "BIgd": nc.dram_tensor("BIgd", [128, NB], BF,
                               kind="ExternalInput").ap(),
        "WOd": nc.dram_tensor("WOd", [64, 1], BF, kind="ExternalInput").ap(),
        "BOd": nc.dram_tensor("BOd", [1, 1], BF, kind="ExternalInput").ap(),
        "y": nc.dram_tensor("y", [NUNITS, 1, 2, NB], BF,
                            kind="ExternalOutput").ap(),
    }
    with tile.TileContext(nc) as tc:
        emit_lstm(tc, aps)
    nc.compile()
    _BUILD_CACHE[key] = nc
    return nc


def make_in_maps(x, W_ih, W_hh, b_ih, b_hh, W_out, b_out):
    bf16 = ml_dtypes.bfloat16
    wd = prep_weights(W_ih, W_hh, b_ih, b_hh, W_out, b_out)
    xt = np.ascontiguousarray(x.transpose(1, 2, 0))   # [T, I, B] f32
    in_maps = []
    for c in range(NCORES):
        sl = xt[:, :, c * B_LOC:(c + 1) * B_LOC]
        # [T, I, B_loc] -> [T/2(chunk), 2(step), I, NU, 2(grp), NB]
        blk = sl.reshape(T // 2, 2, INP, NUNITS, 2, NB)
        xab = np.ascontiguousarray(
            blk.transpose(0, 2, 3, 4, 1, 5)).astype(bf16)
        in_maps.append({"xab": xab, **wd})
    return in_maps


def kernel(x, W_ih, W_hh, b_ih, b_hh, W_out, b_out):
    from concourse.bass_utils import run_bass_kernel_spmd

    nc = build_nc()
    in_maps = make_in_maps(x, W_ih, W_hh, b_ih, b_hh, W_out, b_out)
    res = run_bass_kernel_spmd(nc, in_maps, core_ids=list(range(NCORES)))
    y = np.concatenate([res.results[c]["y"].astype(np.float32).reshape(B_LOC)
                        for c in range(NCORES)])
    return y.reshape(B, 1).astype(np.float32)


# revision 18
# speedup vs baseline: 1.9682x; 1.3201x over previous
"""LSTM (B=131072, T=10, INP=HID=64) + linear head, data-parallel on 8 TRN2 cores.

v7 layout (per core, B_loc=16384, 16 "units" of two 512-col groups A/B):
  - Feature-major: features on SBUF partitions, batch on the free dim. PSUM
    per unit-step: [128, 4, NB] banks (i, f, g, o), bank = [gate_A(0:64);
    gate_B(64:128)], so all elementwise ops run 128 lanes wide.
  - Both groups use rhs layout [h(0:64); x(64:128)] inside one persistent
    tile RAB[128, group, slot(4), NB] per unit, sharing a single weight copy.
    h_A lands aligned; h_B is written with a cross-partition output (legal:
    only tensor-op *inputs* must share a base partition). x is DMA'd two
    steps ahead into slot pairs, so DMA WAR waits are against long-retired
    readers and the SP queue never head-of-line blocks.
  - Bias: banks i, f seeded by K=1 matmuls on PE; bank g by a DVE copy from
    an SBUF image; the o-gate sigmoid is a separate ACT instruction carrying
    its bias as a per-partition vector (free). g weights/bias pre-doubled so
    tanh(g) = 2*sig(2g)-1. tanh(c) batched across unit pairs.
  - Two-phase software pipeline with a LAG-pair lead: phase 1 (matmuls,
    sigmoids, gate products, c update) runs ahead; phase 2 (tanh, h-muls,
    head) for pair q is emitted LAG pairs later, so the ACT queue never
    stalls on the cross-engine elementwise chain.
  - Elementwise: i*g and f*c products on Pool(GPSIMD); Gt fix, g-bank seed,
    c-add, h-muls, head staging on DVE.
"""

import numpy as np
import ml_dtypes

import concourse.bass as bass
import concourse.mybir as mybir
from concourse import bacc
import concourse.tile as tile

HID = 64
INP = 64
T = 10
B = 131072
NCORES = 8
B_LOC = B // NCORES   # 16384
NB = 512              # batch columns per group
NUNITS = B_LOC // (2 * NB)  # 16 units of (A, B) groups
NSLOT = 4             # rhs time slots (2-step DMA chunks, 2-step prefetch)
LAG = 2               # pairs of phase-1 lead over phase 2

BF = mybir.dt.bfloat16
F32 = mybir.dt.float32
AF = mybir.ActivationFunctionType
ALU = mybir.AluOpType

# psum gate-slice order matches torch block order: 0=i, 1=f, 2=g, 3=o
GATE_SCALE = [1.0, 1.0, 2.0, 1.0]


def emit_lstm(tc, aps):
    nc = tc.nc
    xab, Wd, BWd, BIgd, WOd, BOd, y = (
        aps["xab"], aps["Wd"], aps["BWd"], aps["BIgd"],
        aps["WOd"], aps["BOd"], aps["y"])

    with (
        tc.tile_pool(name="const", bufs=1) as cpool,
        tc.tile_pool(name="rhs", bufs=1) as rpool,
        tc.tile_pool(name="cstate", bufs=2) as spool,
        tc.tile_pool(name="gs", bufs=6) as gpool,
        tc.tile_pool(name="work", bufs=4) as wpool,
        tc.tile_pool(name="small", bufs=3) as qpool,
        tc.tile_pool(name="psum", bufs=2, space="PSUM") as ppool,
    ):
        W = cpool.tile([128, 4, 64], BF)    # [k(h;x), slice, m]
        nc.sync.dma_start(out=W, in_=Wd)
        BW = cpool.tile([1, 3, 128], BF)    # i, f, g seed rows
        nc.sync.dma_start(out=BW, in_=BWd)
        BIg = cpool.tile([128, NB], BF)     # o bank bias image
        nc.sync.dma_start(out=BIg, in_=BIgd)
        WO = cpool.tile([64, 1], BF)
        nc.sync.dma_start(out=WO, in_=WOd)
        BO = cpool.tile([1, 1], BF)
        nc.sync.dma_start(out=BO, in_=BOd)
        ones_sb = cpool.tile([1, NB], BF)
        nc.vector.memset(ones_sb, 1.0)

        # persistent rhs tiles: [h(0:64); x(64:128)] x group x slot
        R = [rpool.tile([128, 2, NSLOT, NB], BF, tag=f"r{u}", name=f"r_{u}")
             for u in range(NUNITS)]

        def x_dma(u, chunk):
            sl = (2 * chunk) % NSLOT
            nc.sync.dma_start(out=R[u][64:128, :, sl:sl + 2, :],
                              in_=xab[chunk, :, u])

        for u in range(NUNITS):
            x_dma(u, 0)

        CP = [(None, None)] * (NUNITS // 2)  # (new, old) c tiles per pair
        OO = {}                              # (t, u) -> o-gate sigmoid tile

        def phase1(t, u):
            p = u // 2
            sl = t % NSLOT
            ru = R[u]
            def gate_mms(ps, s0, s1):
                for s in (s0, s1):
                    lst = s == s1
                    if t == 0:
                        nc.tensor.matmul(ps[0:64, s - s0], W[64:128, s, :],
                                         ru[64:128, 0, 0, :], start=False,
                                         stop=False, skip_group_check=True)
                        nc.tensor.matmul(ps[64:128, s - s0], W[64:128, s, :],
                                         ru[64:128, 1, 0, :], start=False,
                                         stop=lst, skip_group_check=True)
                    else:
                        nc.tensor.matmul(ps[0:64, s - s0], W[:, s, :],
                                         ru[:, 0, sl, :], start=False,
                                         stop=False, skip_group_check=True)
                        nc.tensor.matmul(ps[64:128, s - s0], W[:, s, :],
                                         ru[:, 1, sl, :], start=False,
                                         stop=lst, skip_group_check=True)

            ps_if = ppool.tile([128, 2, NB], F32, tag="pif", name=f"pif_{t}_{u}")
            for s in range(2):
                nc.tensor.matmul(ps_if[:, s], BW[:, s, :], ones_sb,
                                 start=True, stop=False,
                                 skip_group_check=True)
            gate_mms(ps_if, 0, 1)
            GSif = gpool.tile([128, 2, NB], BF, tag="GSif", name=f"gsif_{t}_{u}")
            nc.scalar.activation(GSif, ps_if, AF.Sigmoid)

            ps_go = ppool.tile([128, 2, NB], F32, tag="pgo", name=f"pgo_{t}_{u}")
            nc.tensor.matmul(ps_go[:, 0], BW[:, 2, :], ones_sb,
                             start=True, stop=False, skip_group_check=True)
            nc.vector.tensor_copy(out=ps_go[:, 1, :], in_=BIg)
            gate_mms(ps_go, 2, 3)
            GS = gpool.tile([128, 2, NB], BF, tag="GSgo", name=f"gsgo_{t}_{u}")
            nc.scalar.activation(GS, ps_go, AF.Sigmoid)
            OO[(t, u)] = GS

            if u % 2 == 0:
                CPn = spool.tile([128, 2, NB], BF, tag=f"C{p}",
                                 name=f"c_{t}_{p}")
                CP[p] = (CPn, CP[p][0])
            CPnew, CPold = CP[p]
            if t != 0:
                # f*c needs only the first sigmoid: start it early
                ww = qpool.tile([128, NB], BF, tag="ww", name=f"ww_{t}_{u}")
                nc.vector.tensor_mul(ww, GSif[:, 1], CPold[:, u % 2, :])
            Gt = qpool.tile([128, NB], BF, tag="Gt", name=f"gt_{t}_{u}")
            nc.gpsimd.tensor_scalar(Gt, GS[:, 0], 2.0, -1.0,
                                    ALU.mult, ALU.add)
            uu = qpool.tile([128, NB], BF, tag="uu", name=f"uu_{t}_{u}")
            nc.gpsimd.tensor_mul(uu, GSif[:, 0], Gt)
            if t == 0:
                nc.vector.tensor_copy(out=CPnew[:, u % 2, :], in_=uu)
            else:
                nc.vector.tensor_add(CPnew[:, u % 2, :], uu, ww)

        def phase2(t, q):
            last = t == T - 1
            TT = wpool.tile([128, 2, NB], BF, tag="TT", name=f"tt_{t}_{q}")
            nc.scalar.activation(TT, CP[q][0], AF.Tanh)
            for v in (2 * q, 2 * q + 1):
                O = OO.pop((t, v))[:, 1]
                if not last:
                    if t % 2 == 0 and t + 2 < T:
                        x_dma(v, (t + 2) // 2)
                    sln = (t + 1) % NSLOT
                    rv = R[v]
                    nc.vector.tensor_mul(rv[0:64, 0, sln, :],
                                         O[0:64, :], TT[0:64, v % 2])
                    nc.vector.tensor_mul(rv[0:64, 1, sln, :],
                                         O[64:128, :], TT[64:128, v % 2])
                else:
                    H = wpool.tile([64, 2, NB], BF, tag="TT", name=f"h_{v}")
                    nc.vector.tensor_mul(H[:, 0, :], O[0:64, :],
                                         TT[0:64, v % 2])
                    nc.vector.tensor_mul(H[:, 1, :], O[64:128, :],
                                         TT[64:128, v % 2])
                    ob = wpool.tile([1, 2, NB], BF, tag="TT", name=f"ob_{v}")
                    for g in range(2):
                        op = ppool.tile([1, NB], F32, tag="pif",
                                        name=f"op_{v}_{g}")
                        nc.tensor.matmul(op, BO, ones_sb,
                                         start=True, stop=False,
                                         skip_group_check=True)
                        nc.tensor.matmul(op, WO, H[:, g, :],
                                         start=False, stop=True,
                                         skip_group_check=True)
                        nc.vector.tensor_copy(out=ob[:, g, :], in_=op)
                    nc.sync.dma_start(out=y[v], in_=ob)

        pending = []
        for t in range(T):
            for u in range(NUNITS):
                phase1(t, u)
                if u % 2 == 1:
                    pending.append((t, u // 2))
                    if len(pending) > LAG:
                        phase2(*pending.pop(0))
        while pending:
            phase2(*pending.pop(0))


def prep_weights(W_ih, W_hh, b_ih, b_hh, W_out, b_out):
    """Host-side packing (numpy). Returns DRAM arrays for the kernel."""
    bf16 = ml_dtypes.bfloat16
    W = np.zeros((128, 4, 64), np.float32)      # rhs layout [h; x]
    BIAS = np.zeros((4, 128), np.float32)
    b = (b_ih + b_hh).astype(np.float32)
    for s in range(4):
        blk_ih = W_ih[s * 64:(s + 1) * 64, :].astype(np.float32)
        blk_hh = W_hh[s * 64:(s + 1) * 64, :].astype(np.float32)
        scale = GATE_SCALE[s]
        W[0:64, s, :] = blk_hh.T * scale
        W[64:128, s, :] = blk_ih.T * scale
        bb = b[s * 64:(s + 1) * 64] * scale
        BIAS[s, 0:64] = bb
        BIAS[s, 64:128] = bb
    BW = BIAS[0:3][None, :, :]                   # [1, 3, 128] i, f, g seeds
    BIg = np.broadcast_to(BIAS[3][:, None], (128, NB))
    WO = W_out[0].astype(np.float32).reshape(64, 1)
    BO = np.full((1, 1), np.float32(b_out[0]))
    return {
        "Wd": W.astype(bf16),
        "BWd": np.ascontiguousarray(BW).astype(bf16),
        "BIgd": np.ascontiguousarray(BIg).astype(bf16),
        "WOd": WO.astype(bf16),
        "BOd": BO.astype(bf16),
    }


_BUILD_CACHE = {}


def build_nc():
    key = "nc_v7"
    if key in _BUILD_CACHE:
        return _BUILD_CACHE[key]
    nc = bacc.Bacc("TRN2", target_bir_lowering=False, debug=False)
    aps = {
        "xab": nc.dram_tensor("xab", [T // 2, INP, NUNITS, 2, 2, NB], BF,
                              kind="ExternalInput").ap(),
        "Wd": nc.dram_tensor("Wd", [128, 4, 64], BF,
                             kind="ExternalInput").ap(),
        "BWd": nc.dram_tensor("BWd", [1, 3, 128], BF, kind="ExternalInput").ap(),
        "BIgd": nc.dram_tensor("BIgd", [128, NB], BF,
                               kind="ExternalInput").ap(),
        "WOd": nc.dram_tensor("WOd", [64, 1], BF, kind="ExternalInput").ap(),
        "BOd": nc.dram_tensor("BOd", [1, 1], BF, kind="ExternalInput").ap(),
        "y": nc.dram_tensor("y", [NUNITS, 1, 2, NB], BF,
                            kind="ExternalOutput").ap(),
    }
    with tile.TileContext(nc) as tc:
        emit_lstm(tc, aps)
    nc.compile()
    _BUILD_CACHE[key] = nc
    return nc


def make_in_maps(x, W_ih, W_hh, b_ih, b_hh, W_out, b_out):
    bf16 = ml_dtypes.bfloat16
    wd = prep_weights(W_ih, W_hh, b_ih, b_hh, W_out, b_out)
    xt = np.ascontiguousarray(x.transpose(1, 2, 0))   # [T, I, B] f32
    in_maps = []
    for c in range(NCORES):
        sl = xt[:, :, c * B_LOC:(c + 1) * B_LOC]
        # [T, I, B_loc] -> [T/2(chunk), 2(step), I, NU, 2(grp), NB]
        blk = sl.reshape(T // 2, 2, INP, NUNITS, 2, NB)
        xab = np.ascontiguousarray(
            blk.transpose(0, 2, 3, 4, 1, 5)).astype(bf16)
        in_maps.append({"xab": xab, **wd})
    return in_maps


def kernel(x, W_ih, W_hh, b_ih, b_hh, W_out, b_out):
    from concourse.bass_utils import run_bass_kernel_spmd

    nc = build_nc()
    in_maps = make_in_maps(x, W_ih, W_hh, b_ih, b_hh, W_out, b_out)
    res = run_bass_kernel_spmd(nc, in_maps, core_ids=list(range(NCORES)))
    y = np.concatenate([res.results[c]["y"].astype(np.float32).reshape(B_LOC)
                        for c in range(NCORES)])
    return y.reshape(B, 1).astype(np.float32)


# revision 19
# speedup vs baseline: 2.0713x; 1.0524x over previous
"""LSTM (B=131072, T=10, INP=HID=64) + linear head, data-parallel on 8 TRN2 cores.

v7 layout (per core, B_loc=16384, 16 "units" of two 512-col groups A/B):
  - Feature-major: features on SBUF partitions, batch on the free dim. PSUM
    per unit-step: [128, 4, NB] banks (i, f, g, o), bank = [gate_A(0:64);
    gate_B(64:128)], so all elementwise ops run 128 lanes wide.
  - Both groups use rhs layout [h(0:64); x(64:128)] inside one persistent
    tile RAB[128, group, slot(4), NB] per unit, sharing a single weight copy.
    h_A lands aligned; h_B is written with a cross-partition output (legal:
    only tensor-op *inputs* must share a base partition). x is DMA'd two
    steps ahead into slot pairs, so DMA WAR waits are against long-retired
    readers and the SP queue never head-of-line blocks.
  - Bias: banks i, f seeded by K=1 matmuls on PE; bank g by a DVE copy from
    an SBUF image; the o-gate sigmoid is a separate ACT instruction carrying
    its bias as a per-partition vector (free). g weights/bias pre-doubled so
    tanh(g) = 2*sig(2g)-1. tanh(c) batched across unit pairs.
  - Two-phase software pipeline with a LAG-pair lead: phase 1 (matmuls,
    sigmoids, gate products, c update) runs ahead; phase 2 (tanh, h-muls,
    head) for pair q is emitted LAG pairs later, so the ACT queue never
    stalls on the cross-engine elementwise chain.
  - Elementwise: i*g and f*c products on Pool(GPSIMD); Gt fix, g-bank seed,
    c-add, h-muls, head staging on DVE.
"""

import numpy as np
import ml_dtypes

import concourse.bass as bass
import concourse.mybir as mybir
from concourse import bacc
import concourse.tile as tile

HID = 64
INP = 64
T = 10
B = 131072
NCORES = 8
B_LOC = B // NCORES   # 16384
NB = 512              # batch columns per group
NUNITS = B_LOC // (2 * NB)  # 16 units of (A, B) groups
NSLOT = 4             # rhs time slots (2-step DMA chunks, 2-step prefetch)
LAG = 2               # pairs of phase-1 lead over phase 2

BF = mybir.dt.bfloat16
F32 = mybir.dt.float32
AF = mybir.ActivationFunctionType
ALU = mybir.AluOpType

# psum gate-slice order matches torch block order: 0=i, 1=f, 2=g, 3=o
GATE_SCALE = [1.0, 1.0, 2.0, 1.0]


def emit_lstm(tc, aps):
    nc = tc.nc
    xab, Wd, BWd, BIgd, WOd, BOd, y = (
        aps["xab"], aps["Wd"], aps["BWd"], aps["BIgd"],
        aps["WOd"], aps["BOd"], aps["y"])

    with (
        tc.tile_pool(name="const", bufs=1) as cpool,
        tc.tile_pool(name="rhs", bufs=1) as rpool,
        tc.tile_pool(name="cstate", bufs=2) as spool,
        tc.tile_pool(name="gs", bufs=6) as gpool,
        tc.tile_pool(name="work", bufs=4) as wpool,
        tc.tile_pool(name="small", bufs=3) as qpool,
        tc.tile_pool(name="psum", bufs=2, space="PSUM") as ppool,
    ):
        W = cpool.tile([128, 4, 64], BF)    # [k(h;x), slice, m]
        nc.sync.dma_start(out=W, in_=Wd)
        BW = cpool.tile([1, 3, 128], BF)    # i, f, g seed rows
        nc.sync.dma_start(out=BW, in_=BWd)
        BIg = cpool.tile([128, NB], BF)     # o bank bias image
        nc.sync.dma_start(out=BIg, in_=BIgd)
        WO = cpool.tile([64, 1], BF)
        nc.sync.dma_start(out=WO, in_=WOd)
        BO = cpool.tile([1, 1], BF)
        nc.sync.dma_start(out=BO, in_=BOd)
        ones_sb = cpool.tile([1, NB], BF)
        nc.vector.memset(ones_sb, 1.0)

        # persistent rhs tiles: [h(0:64); x(64:128)] x group x slot
        R = [rpool.tile([128, 2, NSLOT, NB], BF, tag=f"r{u}", name=f"r_{u}")
             for u in range(NUNITS)]

        def x_dma(u, chunk):
            sl = (2 * chunk) % NSLOT
            nc.sync.dma_start(out=R[u][64:128, :, sl:sl + 2, :],
                              in_=xab[chunk, :, u])

        for u in range(NUNITS):
            x_dma(u, 0)

        CP = [(None, None)] * (NUNITS // 2)  # (new, old) c tiles per pair
        OO = {}                              # (t, u) -> o-gate sigmoid tile

        def phase1(t, u):
            p = u // 2
            sl = t % NSLOT
            ru = R[u]
            def gate_mms(ps, s0, s1):
                for s in (s0, s1):
                    lst = s == s1
                    if t == 0:
                        nc.tensor.matmul(ps[0:64, s - s0], W[64:128, s, :],
                                         ru[64:128, 0, 0, :], start=False,
                                         stop=False, skip_group_check=True)
                        nc.tensor.matmul(ps[64:128, s - s0], W[64:128, s, :],
                                         ru[64:128, 1, 0, :], start=False,
                                         stop=lst, skip_group_check=True)
                    else:
                        nc.tensor.matmul(ps[0:64, s - s0], W[:, s, :],
                                         ru[:, 0, sl, :], start=False,
                                         stop=False, skip_group_check=True)
                        nc.tensor.matmul(ps[64:128, s - s0], W[:, s, :],
                                         ru[:, 1, sl, :], start=False,
                                         stop=lst, skip_group_check=True)

            ps_if = ppool.tile([128, 2, NB], F32, tag="pif", name=f"pif_{t}_{u}")
            for s in range(2):
                nc.tensor.matmul(ps_if[:, s], BW[:, s, :], ones_sb,
                                 start=True, stop=False,
                                 skip_group_check=True)
            gate_mms(ps_if, 0, 1)
            GSif = gpool.tile([128, 2, NB], BF, tag="GSif", name=f"gsif_{t}_{u}")
            nc.scalar.activation(GSif, ps_if, AF.Sigmoid)

            ps_go = ppool.tile([128, 2, NB], F32, tag="pgo", name=f"pgo_{t}_{u}")
            nc.tensor.matmul(ps_go[:, 0], BW[:, 2, :], ones_sb,
                             start=True, stop=False, skip_group_check=True)
            nc.vector.tensor_copy(out=ps_go[:, 1, :], in_=BIg)
            gate_mms(ps_go, 2, 3)
            GS = gpool.tile([128, 2, NB], BF, tag="GSgo", name=f"gsgo_{t}_{u}")
            nc.scalar.activation(GS, ps_go, AF.Sigmoid)
            OO[(t, u)] = GS

            if u % 2 == 0:
                CPn = spool.tile([128, 2, NB], BF, tag=f"C{p}",
                                 name=f"c_{t}_{p}")
                CP[p] = (CPn, CP[p][0])
            CPnew, CPold = CP[p]
            if t != 0:
                # f*c needs only the first sigmoid: start it early, off the
                # critical chain, on Pool
                ww = qpool.tile([128, NB], BF, tag="ww", name=f"ww_{t}_{u}")
                nc.gpsimd.tensor_mul(ww, GSif[:, 1], CPold[:, u % 2, :])
            Gt = qpool.tile([128, NB], BF, tag="Gt", name=f"gt_{t}_{u}")
            nc.vector.tensor_scalar(Gt, GS[:, 0], 2.0, -1.0,
                                    ALU.mult, ALU.add)
            uu = qpool.tile([128, NB], BF, tag="uu", name=f"uu_{t}_{u}")
            nc.vector.tensor_mul(uu, GSif[:, 0], Gt)
            if t == 0:
                nc.vector.tensor_copy(out=CPnew[:, u % 2, :], in_=uu)
            else:
                nc.vector.tensor_add(CPnew[:, u % 2, :], uu, ww)

        def phase2(t, q):
            last = t == T - 1
            TT = wpool.tile([128, 2, NB], BF, tag="TT", name=f"tt_{t}_{q}")
            nc.scalar.activation(TT, CP[q][0], AF.Tanh)
            for v in (2 * q, 2 * q + 1):
                O = OO.pop((t, v))[:, 1]
                if not last:
                    if t % 2 == 0 and t + 2 < T:
                        x_dma(v, (t + 2) // 2)
                    sln = (t + 1) % NSLOT
                    rv = R[v]
                    nc.vector.tensor_mul(rv[0:64, 0, sln, :],
                                         O[0:64, :], TT[0:64, v % 2])
                    nc.vector.tensor_mul(rv[0:64, 1, sln, :],
                                         O[64:128, :], TT[64:128, v % 2])
                else:
                    H = wpool.tile([64, 2, NB], BF, tag="TT", name=f"h_{v}")
                    nc.vector.tensor_mul(H[:, 0, :], O[0:64, :],
                                         TT[0:64, v % 2])
                    nc.vector.tensor_mul(H[:, 1, :], O[64:128, :],
                                         TT[64:128, v % 2])
                    ob = wpool.tile([1, 2, NB], BF, tag="TT", name=f"ob_{v}")
                    for g in range(2):
                        op = ppool.tile([1, NB], F32, tag="pif",
                                        name=f"op_{v}_{g}")
                        nc.tensor.matmul(op, BO, ones_sb,
                                         start=True, stop=False,
                                         skip_group_check=True)
                        nc.tensor.matmul(op, WO, H[:, g, :],
                                         start=False, stop=True,
                                         skip_group_check=True)
                        nc.vector.tensor_copy(out=ob[:, g, :], in_=op)
                    nc.sync.dma_start(out=y[v], in_=ob)

        pending = []
        for t in range(T):
            for u in range(NUNITS):
                phase1(t, u)
                if u % 2 == 1:
                    pending.append((t, u // 2))
                    if len(pending) > LAG:
                        phase2(*pending.pop(0))
        while pending:
            phase2(*pending.pop(0))


def prep_weights(W_ih, W_hh, b_ih, b_hh, W_out, b_out):
    """Host-side packing (numpy). Returns DRAM arrays for the kernel."""
    bf16 = ml_dtypes.bfloat16
    W = np.zeros((128, 4, 64), np.float32)      # rhs layout [h; x]
    BIAS = np.zeros((4, 128), np.float32)
    b = (b_ih + b_hh).astype(np.float32)
    for s in range(4):
        blk_ih = W_ih[s * 64:(s + 1) * 64, :].astype(np.float32)
        blk_hh = W_hh[s * 64:(s + 1) * 64, :].astype(np.float32)
        scale = GATE_SCALE[s]
        W[0:64, s, :] = blk_hh.T * scale
        W[64:128, s, :] = blk_ih.T * scale
        bb = b[s * 64:(s + 1) * 64] * scale
        BIAS[s, 0:64] = bb
        BIAS[s, 64:128] = bb
    BW = BIAS[0:3][None, :, :]                   # [1, 3, 128] i, f, g seeds
    BIg = np.broadcast_to(BIAS[3][:, None], (128, NB))
    WO = W_out[0].astype(np.float32).reshape(64, 1)
    BO = np.full((1, 1), np.float32(b_out[0]))
    return {
        "Wd": W.astype(bf16),
        "BWd": np.ascontiguousarray(BW).astype(bf16),
        "BIgd": np.ascontiguousarray(BIg).astype(bf16),
        "WOd": WO.astype(bf16),
        "BOd": BO.astype(bf16),
    }


_BUILD_CACHE = {}


def build_nc():
    key = "nc_v7"
    if key in _BUILD_CACHE:
        return _BUILD_CACHE[key]
    nc = bacc.Bacc("TRN2", target_bir_lowering=False, debug=False)
    aps = {
        "xab": nc.dram_tensor("xab", [T // 2, INP, NUNITS, 2, 2, NB], BF,
                              kind="ExternalInput").ap(),
        "Wd": nc.dram_tensor("Wd", [128, 4, 64], BF,
                             kind="ExternalInput").ap(),
        "BWd": nc.dram_tensor("BWd", [1, 3, 128], BF, kind="ExternalInput").ap(),
        "BIgd": nc.dram_tensor("BIgd", [128, NB], BF,
                               kind="ExternalInput").ap(),
        "WOd": nc.dram_tensor("WOd", [64, 1], BF, kind="ExternalInput").ap(),
        "BOd": nc.dram_tensor("BOd", [1, 1], BF, kind="ExternalInput").ap(),
        "y": nc.dram_tensor("y", [NUNITS, 1, 2, NB], BF,
                            kind="ExternalOutput").ap(),
    }
    with tile.TileContext(nc) as tc:
        emit_lstm(tc, aps)
    nc.compile()
    _BUILD_CACHE[key] = nc
    return nc


def make_in_maps(x, W_ih, W_hh, b_ih, b_hh, W_out, b_out):
    bf16 = ml_dtypes.bfloat16
    wd = prep_weights(W_ih, W_hh, b_ih, b_hh, W_out, b_out)
    xt = np.ascontiguousarray(x.transpose(1, 2, 0))   # [T, I, B] f32
    in_maps = []
    for c in range(NCORES):
        sl = xt[:, :, c * B_LOC:(c + 1) * B_LOC]
        # [T, I, B_loc] -> [T/2(chunk), 2(step), I, NU, 2(grp), NB]
        blk = sl.reshape(T // 2, 2, INP, NUNITS, 2, NB)
        xab = np.ascontiguousarray(
            blk.transpose(0, 2, 3, 4, 1, 5)).astype(bf16)
        in_maps.append({"xab": xab, **wd})
    return in_maps


def kernel(x, W_ih, W_hh, b_ih, b_hh, W_out, b_out):
    from concourse.bass_utils import run_bass_kernel_spmd

    nc = build_nc()
    in_maps = make_in_maps(x, W_ih, W_hh, b_ih, b_hh, W_out, b_out)
    res = run_bass_kernel_spmd(nc, in_maps, core_ids=list(range(NCORES)))
    y = np.concatenate([res.results[c]["y"].astype(np.float32).reshape(B_LOC)
                        for c in range(NCORES)])
    return y.reshape(B, 1).astype(np.float32)
